# revision 3
# baseline (speedup 1.0000x reference)
"""Cross-modal triplet loss (margin ranking on hardest pos/neg pairs) on 8 trn2 NeuronCores.

Strategy: shard rows of modal1 across the 8 cores (512 rows each); replicate
modal2 and targets. Inputs are quantized to fp8 e4m3 on the host and shipped in
two layouts: K-major DoubleRow layout (two 128-row K-subtiles side by side) so
the PE runs fp8 DoubleRow matmuls at 2x bf16 rate with no on-chip transposes,
and natural-layout bf16 (exact widening of the same fp8 values) for row-norm
computation via scalar-engine Square+accumulate.

Per (m-tile, chunk) PSUM group:
    psum[m, j] = dot(m1q[m], m2q[j]) - sq2[j]/2 - (BIG/2) * mask[m, j]
computed as one bf16 "aug" matmul (66 contraction rows: same-identity one-hot
mask over 64 ids, and the hi/lo bf16 split of -sq2/2) followed by 8 fp8
DoubleRow matmuls (K=2048). The aug matmul leads the group so the group close
never waits on sq2 production, which is software-pipelined one chunk ahead.
h = -2*psum = sq2 - 2g + BIG*mask, so the row-wise psum max/min give
hardest-negative / (BIG + hardest-positive) squared distances up to the row
constant sq1[m], added after the reductions in fp32. sqrt only on the final
per-row values. Per-core loss/precision partials are combined on the host.

Distances are exact metrics on the fp8-quantized vectors (norms computed from
the same quantized values the matmul sees): loss error ~7e-4 relative vs the
2e-2 gate; precision stays exactly 0 (min row gap an-ap ~ 4.2 >> fp8 noise).
"""

import functools

import ml_dtypes
import numpy as np

import concourse.bass as bass
import concourse.mybir as mybir
import concourse.tile as tile
from concourse import bacc
from concourse.bass_utils import run_bass_kernel_spmd

F32 = mybir.dt.float32
BF16 = mybir.dt.bfloat16
FP8 = mybir.dt.float8e4
E4 = ml_dtypes.float8_e4m3
BF = ml_dtypes.bfloat16
OP = mybir.AluOpType
AF = mybir.ActivationFunctionType
AX = mybir.AxisListType.X
DRM = mybir.MatmulPerfMode.DoubleRow

N, D, NIDS, P = 4096, 2048, 64, 128
NCORES = 8
SH = N // NCORES      # 512 rows of modal1 per core
MT = SH // P          # 4 m-tiles per core
KS = D // 256         # 8 DoubleRow K-supertiles (256 contraction rows each)
CHUNK = 512           # modal2 columns per chunk (one PSUM bank of fp32)
NJC = N // CHUNK      # 8 chunks
JTC = CHUNK // P      # 4 j-tiles per chunk
KAUG = 66             # one-hot mask (0:64), sq2 hi/lo (64:66)
BIG = 16384.0         # > max dist_sq (~6500); exact in bf16
EPS = 1e-12


def _build(margin: float) -> bass.Bass:
    nc = bacc.Bacc(num_swdge_queues=4)
    m1r_d = nc.dram_tensor("m1r", [KS * P, MT * 256], FP8, kind="ExternalInput")
    m2r_d = nc.dram_tensor("m2r", [KS * P, NJC * 1024], FP8, kind="ExternalInput")
    m1n_d = nc.dram_tensor("m1n", [SH, D], BF16, kind="ExternalInput")
    m2n_d = nc.dram_tensor("m2n", [N, D], BF16, kind="ExternalInput")
    tgt_d = nc.dram_tensor("tgt", [1, N], F32, kind="ExternalInput")
    tgts_d = nc.dram_tensor("tgts", [1, SH], F32, kind="ExternalInput")
    iden_d = nc.dram_tensor("iden", [P, P], F32, kind="ExternalInput")
    iota_d = nc.dram_tensor("iota", [NIDS, 1], F32, kind="ExternalInput")
    out_d = nc.dram_tensor("out", [2 * MT, 1], F32, kind="ExternalOutput")

    with tile.TileContext(nc) as tc:
        with (
            tc.tile_pool(name="const", bufs=1) as const,
            tc.tile_pool(name="m1rp", bufs=KS) as m1rp,
            tc.tile_pool(name="m2rp", bufs=2 * KS) as m2rp,
            tc.tile_pool(name="natp", bufs=8) as natp,
            tc.tile_pool(name="scr", bufs=2) as scrp,
            tc.tile_pool(name="small", bufs=12) as smallp,
            tc.tile_pool(name="stat", bufs=2 * MT + 10) as statp,
            tc.tile_pool(name="psD", bufs=6, space=bass.MemorySpace.PSUM) as psD,
            tc.tile_pool(name="psS", bufs=2, space=bass.MemorySpace.PSUM) as psS,
        ):
            # ---- constants ----
            iden = const.tile([P, P], F32)
            nc.sync.dma_start(iden[:], iden_d[:, :])
            iota_f = const.tile([NIDS, 1], F32)
            nc.sync.dma_start(iota_f[:], iota_d[:, :])
            ones_col = const.tile([P, 1], F32)
            nc.vector.memset(ones_col[:], 1.0)

            # ---- aug lhsT [66, SH]: -BIG/2 * onehot(own ids); rows 64:66 = 1 ----
            laug = const.tile([KAUG, SH], BF16)
            bc1 = const.tile([NIDS, SH], F32)
            nc.sync.dma_start(bc1[:], tgts_d[0:1, :].broadcast_to((NIDS, SH)))
            nc.vector.tensor_scalar(
                laug[0:NIDS, :], bc1[:], iota_f[:], -BIG / 2.0, OP.is_equal, OP.mult
            )
            nc.gpsimd.memset(laug[NIDS:KAUG, :], 1.0)

            # ---- aug rhs [66, N]: onehot(all ids); rows 64:66 filled per chunk ----
            raug = const.tile([KAUG, N], BF16)
            bc2 = const.tile([NIDS, N], F32)
            nc.sync.dma_start(bc2[:], tgt_d[0:1, :].broadcast_to((NIDS, N)))
            nc.vector.tensor_scalar(
                raug[0:NIDS, :], bc2[:], iota_f[:], None, OP.is_equal
            )

            # ---- m1 DR-layout supertiles + sq1 from natural bf16 shard ----
            m1r = []
            for ks in range(KS):
                t = m1rp.tile([P, MT, 2, P], FP8, tag="m1r", name=f"m1r{ks}")
                nc.sync.dma_start(t[:], m1r_d[ks * P : (ks + 1) * P, :])
                m1r.append(t)

            sq1c = const.tile([P, MT], F32)
            for mt in range(MT):
                t = natp.tile([P, D], BF16, tag="m1nat")
                nc.sync.dma_start(t[:], m1n_d[mt * P : (mt + 1) * P, :])
                scr = scrp.tile([P, D], BF16, tag="scr")
                nc.scalar.activation(
                    scr[:], t[:], AF.Square, accum_out=sq1c[:, mt : mt + 1]
                )

            # ---- running per-row min/max of psum over chunks ----
            minb = [statp.tile([P, NJC], F32, tag="stat", name=f"minb{i}") for i in range(MT)]
            maxb = [statp.tile([P, NJC], F32, tag="stat", name=f"maxb{i}") for i in range(MT)]

            # ---- software-pipelined sq2 production (one chunk ahead) ----
            # phase A: DMA natural bf16 j-tiles, Square+accum, hi/lo split
            # phase B (after current chunk's matmuls in the PE queue):
            #          PE-transpose hi/lo into raug rows 64:66
            def sq2_a(jc):
                hls = []
                for jt in range(JTC):
                    j0 = jc * JTC + jt
                    t = natp.tile([P, D], BF16, tag="m2nat")
                    nc.gpsimd.dma_start(t[:], m2n_d[j0 * P : (j0 + 1) * P, :])
                    scr = scrp.tile([P, D], BF16, tag="scr")
                    s2c = smallp.tile([P, 1], F32, tag="sqc")
                    nc.scalar.activation(scr[:], t[:], AF.Square, accum_out=s2c[:])
                    v = smallp.tile([P, 1], F32, tag="sqv")
                    nc.vector.tensor_scalar(v[:], s2c[:], -0.5, None, OP.mult)
                    hb = smallp.tile([P, 1], BF16, tag="hb")
                    nc.vector.tensor_copy(hb[:], v[:])
                    hl = smallp.tile([P, 2], F32, tag="hl")
                    nc.vector.tensor_copy(hl[:, 0:1], hb[:])
                    nc.vector.tensor_sub(hl[:, 1:2], v[:], hl[:, 0:1])
                    hls.append(hl)
                return hls

            def sq2_b(jc, hls):
                for jt in range(JTC):
                    j0 = jc * JTC + jt
                    pS = psS.tile([2, P], F32, tag="psS")
                    nc.tensor.transpose(pS[:], hls[jt][:], iden[:])
                    nc.vector.tensor_copy(
                        raug[NIDS : NIDS + 2, j0 * P : (j0 + 1) * P], pS[:]
                    )

            def m2r_load(jc):
                tiles = []
                for ks in range(KS):
                    t = m2rp.tile([P, 2, CHUNK], FP8, tag="m2r")
                    nc.sync.dma_start(
                        t[:],
                        m2r_d[ks * P : (ks + 1) * P, jc * 1024 : (jc + 1) * 1024],
                    )
                    tiles.append(t)
                return tiles

            def reduce(mt, pdt, jc_):
                nc.vector.tensor_reduce(
                    minb[mt][:, jc_ : jc_ + 1], pdt[:], AX, OP.min
                )
                nc.vector.tensor_reduce(
                    maxb[mt][:, jc_ : jc_ + 1], pdt[:], AX, OP.max
                )

            # preamble: chunk 0 inputs + sq2, then fill raug rows for chunk 0
            m2r_cur = m2r_load(0)
            hls = sq2_a(0)
            sq2_b(0, hls)

            pending_red = []
            for jc in range(NJC):
                # prefetch next chunk's rhs + start its sq2 production
                if jc + 1 < NJC:
                    m2r_next = m2r_load(jc + 1)
                    hls = sq2_a(jc + 1)

                for mt in range(MT):
                    if len(pending_red) >= 2:
                        pending_red.pop(0)()
                    pdt = psD.tile([P, CHUNK], F32, tag="psD")
                    nc.tensor.matmul(
                        pdt[:],
                        laug[:, mt * P : (mt + 1) * P],
                        raug[:, jc * CHUNK : (jc + 1) * CHUNK],
                        start=True,
                        stop=False,
                    )
                    for ks in range(KS):
                        nc.tensor.matmul(
                            pdt[:],
                            m1r[ks][:, mt],
                            m2r_cur[ks][:],
                            start=False,
                            stop=(ks == KS - 1),
                            perf_mode=DRM,
                        )
                    pending_red.append(
                        lambda mt_=mt, pdt_=pdt, jc_=jc: reduce(mt_, pdt_, jc_)
                    )

                # next chunk's sq2 rows enter raug after this chunk's matmuls
                # so the PE queue never stalls on them mid-chunk
                if jc + 1 < NJC:
                    sq2_b(jc + 1, hls)
                    m2r_cur = m2r_next

            for r in pending_red:
                r()

            # ---- finale: per-row ap/an, loss, precision; column sums ----
            pmin = statp.tile([P, MT], F32, tag="fin")
            pmax = statp.tile([P, MT], F32, tag="fin")
            for mt in range(MT):
                nc.vector.tensor_reduce(
                    pmin[:, mt : mt + 1], minb[mt][:], AX, OP.min
                )
                nc.vector.tensor_reduce(
                    pmax[:, mt : mt + 1], maxb[mt][:], AX, OP.max
                )
            # ap_sq = max(-2*pmin - BIG + sq1, EPS); an_sq = max(-2*pmax + sq1, EPS)
            apq = statp.tile([P, MT], F32, tag="fin")
            nc.vector.tensor_scalar(apq[:], pmin[:], -2.0, BIG, OP.mult, OP.subtract)
            nc.vector.tensor_tensor(apq[:], apq[:], sq1c[:], OP.add)
            apq2 = statp.tile([P, MT], F32, tag="fin")
            nc.vector.tensor_scalar(apq2[:], apq[:], EPS, None, OP.max)
            anq = statp.tile([P, MT], F32, tag="fin")
            nc.vector.tensor_scalar(anq[:], pmax[:], -2.0, None, OP.mult)
            nc.vector.tensor_tensor(anq[:], anq[:], sq1c[:], OP.add)
            nc.vector.tensor_scalar(anq[:], anq[:], EPS, None, OP.max)

            prec = statp.tile([P, MT], F32, tag="fin")
            nc.vector.tensor_tensor(prec[:], anq[:], apq2[:], OP.is_gt)

            ap = statp.tile([P, MT], F32, tag="fin")
            nc.scalar.activation(ap[:], apq2[:], AF.Sqrt)
            an = statp.tile([P, MT], F32, tag="fin")
            nc.scalar.activation(an[:], anq[:], AF.Sqrt)

            lp = statp.tile([P, 2 * MT], F32, tag="fin2")
            nc.vector.tensor_sub(lp[:, 0:MT], ap[:], an[:])
            nc.vector.tensor_scalar(
                lp[:, 0:MT], lp[:, 0:MT], margin, 0.0, OP.add, OP.max
            )
            nc.vector.tensor_copy(lp[:, MT : 2 * MT], prec[:])

            pf = psS.tile([2 * MT, 1], F32, tag="psS")
            nc.tensor.matmul(pf[:], lp[:], ones_col[:])
            osb = statp.tile([2 * MT, 1], F32, tag="fin")
            nc.vector.tensor_copy(osb[:], pf[:])
            nc.sync.dma_start(out_d[:, :], osb[:])

    nc.finalize()
    return nc


@functools.lru_cache(maxsize=4)
def _get_program(margin: float) -> bass.Bass:
    return _build(margin)


def _make_in_maps(m1q, m2q, tgt_f32):
    iden = np.eye(P, dtype=np.float32)
    iota = np.arange(NIDS, dtype=np.float32).reshape(NIDS, 1)
    # m2 DR layout: [ks*128+p, jc*1024 + i*512 + jj] = m2q[jc*512+jj, ks*256+i*128+p]
    m2r = np.ascontiguousarray(
        m2q.T.reshape(KS, 2, P, NJC, CHUNK)
        .transpose(0, 2, 3, 1, 4)
        .reshape(KS * P, NJC * 1024)
    )
    m2n = m2q.astype(BF)
    maps = []
    for c in range(NCORES):
        r = m1q[c * SH : (c + 1) * SH]
        # m1 DR layout: [ks*128+p, mt*256 + i*128 + m] = r[mt*128+m, ks*256+i*128+p]
        m1r = np.ascontiguousarray(
            r.T.reshape(KS, 2, P, MT, P)
            .transpose(0, 2, 3, 1, 4)
            .reshape(KS * P, MT * 256)
        )
        maps.append(
            {
                "m1r": m1r,
                "m2r": m2r,
                "m1n": r.astype(BF),
                "m2n": m2n,
                "tgt": tgt_f32,
                "tgts": np.ascontiguousarray(tgt_f32[:, c * SH : (c + 1) * SH]),
                "iden": iden,
                "iota": iota,
            }
        )
    return maps


def run(modal1_inputs, modal2_inputs, targets, margin, trace=False):
    m1q = np.asarray(modal1_inputs, dtype=np.float32).astype(E4)
    m2q = np.asarray(modal2_inputs, dtype=np.float32).astype(E4)
    tgt_f32 = np.asarray(targets).astype(np.float32).reshape(1, N)
    nc = _get_program(float(margin))
    res = run_bass_kernel_spmd(
        nc, _make_in_maps(m1q, m2q, tgt_f32), list(range(NCORES)), trace=trace
    )
    loss_sum = 0.0
    prec_sum = 0.0
    for r in res.results:
        o = r["out"].reshape(-1)
        loss_sum += float(o[:MT].sum())
        prec_sum += float(o[MT:].sum())
    loss = np.float32(loss_sum / N)
    prec = np.float32(prec_sum / N)
    return (loss, prec), res


def kernel(modal1_inputs, modal2_inputs, targets, margin):
    (loss, prec), _ = run(modal1_inputs, modal2_inputs, targets, margin)
    return loss, prec


# revision 6
# speedup vs baseline: 1.0530x; 1.0530x over previous
"""Cross-modal triplet loss (margin ranking on hardest pos/neg pairs) on 8 trn2 NeuronCores.

Strategy: shard rows of modal1 across the 8 cores (512 rows each); replicate
modal2 and targets. Inputs are quantized to fp8 e4m3 on the host and shipped in
two layouts: K-major DoubleRow layout (two 128-row K-subtiles side by side) so
the PE runs fp8 DoubleRow matmuls at 2x bf16 rate with no on-chip transposes,
and natural-layout bf16 (exact widening of the same fp8 values) for row-norm
computation via scalar-engine Square+accumulate.

Per (m-tile, chunk) PSUM group:
    psum[m, j] = dot(m1q[m], m2q[j]) - sq2[j]/2 - (BIG/2) * mask[m, j]
computed as one bf16 "aug" matmul (66 contraction rows: same-identity one-hot
mask over 64 ids, and the hi/lo bf16 split of -sq2/2) followed by 8 fp8
DoubleRow matmuls (K=2048). The aug matmul leads the group so the group close
never waits on sq2 production, which is software-pipelined one chunk ahead.
h = -2*psum = sq2 - 2g + BIG*mask, so the row-wise psum max/min give
hardest-negative / (BIG + hardest-positive) squared distances up to the row
constant sq1[m], added after the reductions in fp32. sqrt only on the final
per-row values. Per-core loss/precision partials are combined on the host.

Distances are exact metrics on the fp8-quantized vectors (norms computed from
the same quantized values the matmul sees): loss error ~7e-4 relative vs the
2e-2 gate; precision stays exactly 0 (min row gap an-ap ~ 4.2 >> fp8 noise).
"""

import functools

import ml_dtypes
import numpy as np

import concourse.bass as bass
import concourse.mybir as mybir
import concourse.tile as tile
from concourse import bacc
from concourse.bass_utils import run_bass_kernel_spmd

F32 = mybir.dt.float32
BF16 = mybir.dt.bfloat16
FP8 = mybir.dt.float8e4
E4 = ml_dtypes.float8_e4m3
BF = ml_dtypes.bfloat16
OP = mybir.AluOpType
AF = mybir.ActivationFunctionType
AX = mybir.AxisListType.X
DRM = mybir.MatmulPerfMode.DoubleRow

N, D, NIDS, P = 4096, 2048, 64, 128
NCORES = 8
SH = N // NCORES      # 512 rows of modal1 per core
MT = SH // P          # 4 m-tiles per core
KS = D // 256         # 8 DoubleRow K-supertiles (256 contraction rows each)
CHUNK = 512           # modal2 columns per chunk (one PSUM bank of fp32)
NJC = N // CHUNK      # 8 chunks
JTC = CHUNK // P      # 4 j-tiles per chunk
KAUG = 66             # one-hot mask (0:64), sq2 hi/lo (64:66)
BIG = 16384.0         # > max dist_sq (~6500); exact in bf16
EPS = 1e-12


def _build(margin: float) -> bass.Bass:
    nc = bacc.Bacc(num_swdge_queues=4)
    m1r_d = nc.dram_tensor("m1r", [KS * P, MT * 256], FP8, kind="ExternalInput")
    m2r_d = nc.dram_tensor("m2r", [KS * P, NJC * 1024], FP8, kind="ExternalInput")
    m1n_d = nc.dram_tensor("m1n", [SH, D], FP8, kind="ExternalInput")
    m2n_d = nc.dram_tensor("m2n", [N, D], FP8, kind="ExternalInput")
    tgt_d = nc.dram_tensor("tgt", [1, N], F32, kind="ExternalInput")
    tgts_d = nc.dram_tensor("tgts", [1, SH], F32, kind="ExternalInput")
    iden_d = nc.dram_tensor("iden", [P, P], F32, kind="ExternalInput")
    iota_d = nc.dram_tensor("iota", [NIDS, 1], F32, kind="ExternalInput")
    out_d = nc.dram_tensor("out", [2 * MT, 1], F32, kind="ExternalOutput")

    with tile.TileContext(nc) as tc:
        with (
            tc.tile_pool(name="const", bufs=1) as const,
            tc.tile_pool(name="m1rp", bufs=KS) as m1rp,
            tc.tile_pool(name="m2rp", bufs=2 * KS) as m2rp,
            tc.tile_pool(name="natp", bufs=8) as natp,
            tc.tile_pool(name="scr", bufs=2) as scrp,
            tc.tile_pool(name="small", bufs=12) as smallp,
            tc.tile_pool(name="stat", bufs=2 * MT + 10) as statp,
            tc.tile_pool(name="psD", bufs=7, space=bass.MemorySpace.PSUM) as psD,
            tc.tile_pool(name="psS", bufs=1, space=bass.MemorySpace.PSUM) as psS,
        ):
            # ---- constants ----
            iden = const.tile([P, P], F32)
            nc.sync.dma_start(iden[:], iden_d[:, :])
            iota_f = const.tile([NIDS, 1], F32)
            nc.sync.dma_start(iota_f[:], iota_d[:, :])
            ones_col = const.tile([P, 1], F32)
            nc.vector.memset(ones_col[:], 1.0)

            # ---- aug lhsT [66, SH]: -BIG/2 * onehot(own ids); rows 64:66 = 1 ----
            laug = const.tile([KAUG, SH], BF16)
            bc1 = const.tile([NIDS, SH], F32)
            nc.sync.dma_start(bc1[:], tgts_d[0:1, :].broadcast_to((NIDS, SH)))
            nc.vector.tensor_scalar(
                laug[0:NIDS, :], bc1[:], iota_f[:], -BIG / 2.0, OP.is_equal, OP.mult
            )
            nc.gpsimd.memset(laug[NIDS:KAUG, :], 1.0)

            # ---- aug rhs [66, N]: onehot(all ids); rows 64:66 filled per chunk ----
            raug = const.tile([KAUG, N], BF16)
            bc2 = const.tile([NIDS, N], F32)
            nc.sync.dma_start(bc2[:], tgt_d[0:1, :].broadcast_to((NIDS, N)))
            nc.vector.tensor_scalar(
                raug[0:NIDS, :], bc2[:], iota_f[:], None, OP.is_equal
            )

            # ---- m1 DR-layout supertiles + sq1 from natural bf16 shard ----
            m1r = []
            for ks in range(KS):
                t = m1rp.tile([P, MT, 2, P], FP8, tag="m1r", name=f"m1r{ks}")
                nc.sync.dma_start(t[:], m1r_d[ks * P : (ks + 1) * P, :])
                m1r.append(t)

            sq1c = const.tile([P, MT], F32)
            for mt in range(MT):
                t = natp.tile([P, D], FP8, tag="m1nat")
                nc.sync.dma_start(t[:], m1n_d[mt * P : (mt + 1) * P, :])
                scr = scrp.tile([P, D], BF16, tag="scr")
                nc.scalar.activation(
                    scr[:], t[:], AF.Square, accum_out=sq1c[:, mt : mt + 1]
                )

            # ---- running per-row min/max of psum over chunks ----
            minb = [statp.tile([P, NJC], F32, tag="stat", name=f"minb{i}") for i in range(MT)]
            maxb = [statp.tile([P, NJC], F32, tag="stat", name=f"maxb{i}") for i in range(MT)]

            # ---- software-pipelined sq2 production (one chunk ahead) ----
            # phase A: DMA natural bf16 j-tiles, Square+accum, hi/lo split
            # phase B (after current chunk's matmuls in the PE queue):
            #          PE-transpose hi/lo into raug rows 64:66
            def sq2_a(jc):
                hls = []
                for jt in range(JTC):
                    j0 = jc * JTC + jt
                    t = natp.tile([P, D], FP8, tag="m2nat")
                    nc.sync.dma_start(t[:], m2n_d[j0 * P : (j0 + 1) * P, :])
                    scr = scrp.tile([P, D], BF16, tag="scr")
                    s2c = smallp.tile([P, 1], F32, tag="sqc")
                    nc.scalar.activation(scr[:], t[:], AF.Square, accum_out=s2c[:])
                    v = smallp.tile([P, 1], F32, tag="sqv")
                    nc.vector.tensor_scalar(v[:], s2c[:], -0.5, None, OP.mult)
                    hb = smallp.tile([P, 1], BF16, tag="hb")
                    nc.vector.tensor_copy(hb[:], v[:])
                    hl = smallp.tile([P, 2], F32, tag="hl")
                    nc.vector.tensor_copy(hl[:, 0:1], hb[:])
                    nc.vector.tensor_sub(hl[:, 1:2], v[:], hl[:, 0:1])
                    hls.append(hl)
                return hls

            def sq2_b(jc, hls):
                for jt in range(JTC):
                    j0 = jc * JTC + jt
                    pS = psS.tile([2, P], F32, tag="psS")
                    nc.tensor.transpose(pS[:], hls[jt][:], iden[:])
                    nc.vector.tensor_copy(
                        raug[NIDS : NIDS + 2, j0 * P : (j0 + 1) * P], pS[:]
                    )

            def m2r_load(jc):
                tiles = []
                for ks in range(KS):
                    t = m2rp.tile([P, 2, CHUNK], FP8, tag="m2r")
                    nc.gpsimd.dma_start(
                        t[:],
                        m2r_d[ks * P : (ks + 1) * P, jc * 1024 : (jc + 1) * 1024],
                    )
                    tiles.append(t)
                return tiles

            def reduce(mt, pdt, jc_):
                nc.vector.tensor_reduce(
                    minb[mt][:, jc_ : jc_ + 1], pdt[:], AX, OP.min
                )
                nc.vector.tensor_reduce(
                    maxb[mt][:, jc_ : jc_ + 1], pdt[:], AX, OP.max
                )

            # preamble: chunk 0 inputs + sq2, then fill raug rows for chunk 0
            m2r_cur = m2r_load(0)
            hls = sq2_a(0)
            sq2_b(0, hls)

            pending_red = []
            for jc in range(NJC):
                # prefetch next chunk's rhs + start its sq2 production
                if jc + 1 < NJC:
                    m2r_next = m2r_load(jc + 1)
                    hls = sq2_a(jc + 1)

                for mt in range(MT):
                    if len(pending_red) >= 2:
                        pending_red.pop(0)()
                    pdt = psD.tile([P, CHUNK], F32, tag="psD")
                    nc.tensor.matmul(
                        pdt[:],
                        laug[:, mt * P : (mt + 1) * P],
                        raug[:, jc * CHUNK : (jc + 1) * CHUNK],
                        start=True,
                        stop=False,
                    )
                    for ks in range(KS):
                        nc.tensor.matmul(
                            pdt[:],
                            m1r[ks][:, mt],
                            m2r_cur[ks][:],
                            start=False,
                            stop=(ks == KS - 1),
                            perf_mode=DRM,
                        )
                    pending_red.append(
                        lambda mt_=mt, pdt_=pdt, jc_=jc: reduce(mt_, pdt_, jc_)
                    )

                # next chunk's sq2 rows enter raug after this chunk's matmuls
                # so the PE queue never stalls on them mid-chunk
                if jc + 1 < NJC:
                    sq2_b(jc + 1, hls)
                    m2r_cur = m2r_next

            for r in pending_red:
                r()

            # ---- finale: per-row ap/an, loss, precision; column sums ----
            pmin = statp.tile([P, MT], F32, tag="fin")
            pmax = statp.tile([P, MT], F32, tag="fin")
            for mt in range(MT):
                nc.vector.tensor_reduce(
                    pmin[:, mt : mt + 1], minb[mt][:], AX, OP.min
                )
                nc.vector.tensor_reduce(
                    pmax[:, mt : mt + 1], maxb[mt][:], AX, OP.max
                )
            # ap_sq = max(-2*pmin - BIG + sq1, EPS); an_sq = max(-2*pmax + sq1, EPS)
            apq = statp.tile([P, MT], F32, tag="fin")
            nc.vector.tensor_scalar(apq[:], pmin[:], -2.0, BIG, OP.mult, OP.subtract)
            nc.vector.tensor_tensor(apq[:], apq[:], sq1c[:], OP.add)
            apq2 = statp.tile([P, MT], F32, tag="fin")
            nc.vector.tensor_scalar(apq2[:], apq[:], EPS, None, OP.max)
            anq = statp.tile([P, MT], F32, tag="fin")
            nc.vector.tensor_scalar(anq[:], pmax[:], -2.0, None, OP.mult)
            nc.vector.tensor_tensor(anq[:], anq[:], sq1c[:], OP.add)
            nc.vector.tensor_scalar(anq[:], anq[:], EPS, None, OP.max)

            prec = statp.tile([P, MT], F32, tag="fin")
            nc.vector.tensor_tensor(prec[:], anq[:], apq2[:], OP.is_gt)

            ap = statp.tile([P, MT], F32, tag="fin")
            nc.scalar.activation(ap[:], apq2[:], AF.Sqrt)
            an = statp.tile([P, MT], F32, tag="fin")
            nc.scalar.activation(an[:], anq[:], AF.Sqrt)

            lp = statp.tile([P, 2 * MT], F32, tag="fin2")
            nc.vector.tensor_sub(lp[:, 0:MT], ap[:], an[:])
            nc.vector.tensor_scalar(
                lp[:, 0:MT], lp[:, 0:MT], margin, 0.0, OP.add, OP.max
            )
            nc.vector.tensor_copy(lp[:, MT : 2 * MT], prec[:])

            pf = psS.tile([2 * MT, 1], F32, tag="psS")
            nc.tensor.matmul(pf[:], lp[:], ones_col[:])
            osb = statp.tile([2 * MT, 1], F32, tag="fin")
            nc.vector.tensor_copy(osb[:], pf[:])
            nc.sync.dma_start(out_d[:, :], osb[:])

    nc.finalize()
    return nc


@functools.lru_cache(maxsize=4)
def _get_program(margin: float) -> bass.Bass:
    return _build(margin)


def _make_in_maps(m1q, m2q, tgt_f32):
    iden = np.eye(P, dtype=np.float32)
    iota = np.arange(NIDS, dtype=np.float32).reshape(NIDS, 1)
    # m2 DR layout: [ks*128+p, jc*1024 + i*512 + jj] = m2q[jc*512+jj, ks*256+i*128+p]
    m2r = np.ascontiguousarray(
        m2q.T.reshape(KS, 2, P, NJC, CHUNK)
        .transpose(0, 2, 3, 1, 4)
        .reshape(KS * P, NJC * 1024)
    )
    maps = []
    for c in range(NCORES):
        r = m1q[c * SH : (c + 1) * SH]
        # m1 DR layout: [ks*128+p, mt*256 + i*128 + m] = r[mt*128+m, ks*256+i*128+p]
        m1r = np.ascontiguousarray(
            r.T.reshape(KS, 2, P, MT, P)
            .transpose(0, 2, 3, 1, 4)
            .reshape(KS * P, MT * 256)
        )
        maps.append(
            {
                "m1r": m1r,
                "m2r": m2r,
                "m1n": r,
                "m2n": m2q,
                "tgt": tgt_f32,
                "tgts": np.ascontiguousarray(tgt_f32[:, c * SH : (c + 1) * SH]),
                "iden": iden,
                "iota": iota,
            }
        )
    return maps


def run(modal1_inputs, modal2_inputs, targets, margin, trace=False):
    m1q = np.asarray(modal1_inputs, dtype=np.float32).astype(E4)
    m2q = np.asarray(modal2_inputs, dtype=np.float32).astype(E4)
    tgt_f32 = np.asarray(targets).astype(np.float32).reshape(1, N)
    nc = _get_program(float(margin))
    res = run_bass_kernel_spmd(
        nc, _make_in_maps(m1q, m2q, tgt_f32), list(range(NCORES)), trace=trace
    )
    loss_sum = 0.0
    prec_sum = 0.0
    for r in res.results:
        o = r["out"].reshape(-1)
        loss_sum += float(o[:MT].sum())
        prec_sum += float(o[MT:].sum())
    loss = np.float32(loss_sum / N)
    prec = np.float32(prec_sum / N)
    return (loss, prec), res


def kernel(modal1_inputs, modal2_inputs, targets, margin):
    (loss, prec), _ = run(modal1_inputs, modal2_inputs, targets, margin)
    return loss, prec


# revision 12
# speedup vs baseline: 1.3331x; 1.2660x over previous
"""Cross-modal triplet loss (margin ranking on hardest pos/neg pairs) on 8 trn2 NeuronCores.

Strategy: shard rows of modal1 across the 8 cores (512 rows each); replicate
modal2 and targets. Inputs are quantized to fp8 e4m3 on the host and shipped in
two layouts: K-major DoubleRow layout (two 128-row K-subtiles side by side) so
the PE runs fp8 DoubleRow matmuls at 2x bf16 rate with no on-chip transposes,
and natural-layout bf16 (exact widening of the same fp8 values) for row-norm
computation via scalar-engine Square+accumulate.

Per (m-tile, chunk) PSUM group:
    psum[m, j] = dot(m1q[m], m2q[j]) - sq2[j]/2 - (BIG/2) * mask[m, j]
computed as one bf16 "aug" matmul (66 contraction rows: same-identity one-hot
mask over 64 ids, and the hi/lo bf16 split of -sq2/2) followed by 8 fp8
DoubleRow matmuls (K=2048). The aug matmul leads the group so the group close
never waits on sq2 production, which is software-pipelined one chunk ahead.
h = -2*psum = sq2 - 2g + BIG*mask, so the row-wise psum max/min give
hardest-negative / (BIG + hardest-positive) squared distances up to the row
constant sq1[m], added after the reductions in fp32. sqrt only on the final
per-row values. Per-core loss/precision partials are combined on the host.

Distances are exact metrics on the fp8-quantized vectors (norms computed from
the same quantized values the matmul sees): loss error ~7e-4 relative vs the
2e-2 gate; precision stays exactly 0 (min row gap an-ap ~ 4.2 >> fp8 noise).
"""

import functools

import ml_dtypes
import numpy as np

import concourse.bass as bass
import concourse.mybir as mybir
import concourse.tile as tile
from concourse import bacc
from concourse.bass_utils import run_bass_kernel_spmd

F32 = mybir.dt.float32
BF16 = mybir.dt.bfloat16
FP8 = mybir.dt.float8e4
E4 = ml_dtypes.float8_e4m3
BF = ml_dtypes.bfloat16
OP = mybir.AluOpType
AF = mybir.ActivationFunctionType
AX = mybir.AxisListType.X
DRM = mybir.MatmulPerfMode.DoubleRow

N, D, NIDS, P = 4096, 2048, 64, 128
NCORES = 8
SH = N // NCORES      # 512 rows of modal1 per core
MT = SH // P          # 4 m-tiles per core
KS = D // 256         # 8 DoubleRow K-supertiles (256 contraction rows each)
CHUNK = 512           # modal2 columns per chunk (one PSUM bank of fp32)
NJC = N // CHUNK      # 8 chunks
JTC = CHUNK // P      # 4 j-tiles per chunk
KAUG = 66             # one-hot mask (0:64), sq2 hi/lo (64:66)
BIG = 16384.0         # > max dist_sq (~6500); exact in bf16
EPS = 1e-12


def _build(margin: float) -> bass.Bass:
    nc = bacc.Bacc(num_swdge_queues=4)
    m1r_d = nc.dram_tensor("m1r", [KS * P, MT * 256], FP8, kind="ExternalInput")
    m2r_d = nc.dram_tensor("m2r", [KS * P, NJC * 1024], FP8, kind="ExternalInput")
    m1n_d = nc.dram_tensor("m1n", [SH, D], FP8, kind="ExternalInput")
    m2n_d = nc.dram_tensor("m2n", [N, D], FP8, kind="ExternalInput")
    tgt_d = nc.dram_tensor("tgt", [1, N], BF16, kind="ExternalInput")
    tgts_d = nc.dram_tensor("tgts", [1, SH], BF16, kind="ExternalInput")
    iden_d = nc.dram_tensor("iden", [P, P], F32, kind="ExternalInput")
    iota_d = nc.dram_tensor("iota", [NIDS, 1], F32, kind="ExternalInput")
    out_d = nc.dram_tensor("out", [2 * MT, 1], F32, kind="ExternalOutput")

    with tile.TileContext(nc) as tc:
        with (
            tc.tile_pool(name="const", bufs=1) as const,
            tc.tile_pool(name="m1rp", bufs=KS) as m1rp,
            tc.tile_pool(name="m2rp", bufs=2 * KS) as m2rp,
            tc.tile_pool(name="natp", bufs=12) as natp,
            tc.tile_pool(name="scr", bufs=4) as scrp,
            tc.tile_pool(name="small", bufs=12) as smallp,
            tc.tile_pool(name="stat", bufs=2 * MT + 10) as statp,
            tc.tile_pool(name="psD", bufs=7, space=bass.MemorySpace.PSUM) as psD,
            tc.tile_pool(name="psS", bufs=1, space=bass.MemorySpace.PSUM) as psS,
        ):
            # ---- constants ----
            iden = const.tile([P, P], F32)
            nc.sync.dma_start(iden[:], iden_d[:, :])
            iota_f = const.tile([NIDS, 1], F32)
            nc.sync.dma_start(iota_f[:], iota_d[:, :])
            ones_col = const.tile([P, 1], F32)
            nc.vector.memset(ones_col[:], 1.0)

            # ---- aug lhsT [66, SH]: -BIG/2 * onehot(own ids); rows 64:66 = 1 ----
            laug = const.tile([KAUG, SH], BF16)
            bc1 = const.tile([NIDS, SH], BF16)
            nc.sync.dma_start(bc1[:], tgts_d[0:1, :].broadcast_to((NIDS, SH)))
            nc.vector.tensor_scalar(
                laug[0:NIDS, :], bc1[:], iota_f[:], -BIG / 2.0, OP.is_equal, OP.mult
            )
            nc.gpsimd.memset(laug[NIDS:KAUG, :], 1.0)

            # ---- aug rhs [66, N]: onehot(all ids); rows 64:66 filled per chunk ----
            raug = const.tile([KAUG, N], BF16)
            bc2 = const.tile([NIDS, N], BF16)
            nc.sync.dma_start(bc2[:], tgt_d[0:1, :].broadcast_to((NIDS, N)))
            nc.vector.tensor_scalar(
                raug[0:NIDS, :], bc2[:], iota_f[:], None, OP.is_equal
            )

            # ---- m1 DR-layout supertiles (gpsimd queues, after chunk-0 rhs) ----
            m1r = []
            for ks in range(KS):
                t = m1rp.tile([P, MT, 2, P], FP8, tag="m1r", name=f"m1r{ks}")
                nc.gpsimd.dma_start(t[:], m1r_d[ks * P : (ks + 1) * P, :])
                m1r.append(t)

            sq1c = const.tile([P, MT], F32)

            def sq1_compute():
                # issued mid-loop: only needed by the finale
                for mt in range(MT):
                    t = natp.tile([P, D], FP8, tag="m1nat")
                    nc.sync.dma_start(t[:], m1n_d[mt * P : (mt + 1) * P, :])
                    scr = scrp.tile([P, D], BF16, tag="scr")
                    nc.scalar.activation(
                        scr[:], t[:], AF.Square, accum_out=sq1c[:, mt : mt + 1]
                    )

            # ---- running per-row min/max of psum over chunks ----
            minb = [statp.tile([P, NJC], F32, tag="stat", name=f"minb{i}") for i in range(MT)]
            maxb = [statp.tile([P, NJC], F32, tag="stat", name=f"maxb{i}") for i in range(MT)]

            # ---- software-pipelined sq2 production (one chunk ahead) ----
            # phase A: DMA natural bf16 j-tiles, Square+accum, hi/lo split
            # phase B (after current chunk's matmuls in the PE queue):
            #          PE-transpose hi/lo into raug rows 64:66
            def sq2_a(jc):
                hls = []
                for jt in range(JTC):
                    j0 = jc * JTC + jt
                    t = natp.tile([P, D], FP8, tag="m2nat")
                    nc.sync.dma_start(t[:], m2n_d[j0 * P : (j0 + 1) * P, :])
                    scr = scrp.tile([P, D], BF16, tag="scr")
                    s2c = smallp.tile([P, 1], F32, tag="sqc")
                    if jt == 3 and jc % 2 == 1:
                        # share the square work with the vector engine
                        nc.vector.tensor_tensor(scr[:], t[:], t[:], OP.mult)
                        nc.vector.tensor_reduce(s2c[:], scr[:], AX, OP.add)
                    else:
                        nc.scalar.activation(
                            scr[:], t[:], AF.Square, accum_out=s2c[:]
                        )
                    v = smallp.tile([P, 1], F32, tag="sqv")
                    nc.vector.tensor_scalar(v[:], s2c[:], -0.5, None, OP.mult)
                    hb = smallp.tile([P, 1], BF16, tag="hb")
                    nc.vector.tensor_copy(hb[:], v[:])
                    hl = smallp.tile([P, 2], F32, tag="hl")
                    nc.vector.tensor_copy(hl[:, 0:1], hb[:])
                    nc.vector.tensor_sub(hl[:, 1:2], v[:], hl[:, 0:1])
                    hls.append(hl)
                return hls

            def sq2_b(jc, hls):
                pS = psS.tile([2, JTC * P], F32, tag="psS")
                for jt in range(JTC):
                    nc.tensor.transpose(
                        pS[:, jt * P : (jt + 1) * P], hls[jt][:], iden[:]
                    )
                nc.vector.tensor_copy(
                    raug[NIDS : NIDS + 2, jc * CHUNK : (jc + 1) * CHUNK], pS[:]
                )

            def m2r_load(jc):
                tiles = []
                for ks in range(KS):
                    t = m2rp.tile([P, 2, CHUNK], FP8, tag="m2r")
                    nc.gpsimd.dma_start(
                        t[:],
                        m2r_d[ks * P : (ks + 1) * P, jc * 1024 : (jc + 1) * 1024],
                    )
                    tiles.append(t)
                return tiles

            def reduce(mt, pdt, jc_):
                nc.vector.tensor_reduce(
                    minb[mt][:, jc_ : jc_ + 1], pdt[:], AX, OP.min
                )
                nc.vector.tensor_reduce(
                    maxb[mt][:, jc_ : jc_ + 1], pdt[:], AX, OP.max
                )

            # preamble: sq2 production runs two chunks ahead of consumption;
            # raug transposes land one chunk ahead (between matmul blocks)
            m2r_tiles = {0: m2r_load(0)}
            hls_q = {0: sq2_a(0), 1: sq2_a(1)}
            sq2_b(0, hls_q.pop(0))

            pending_red = []
            for jc in range(NJC):
                if jc + 1 < NJC:
                    m2r_tiles[jc + 1] = m2r_load(jc + 1)
                if jc + 2 < NJC:
                    hls_q[jc + 2] = sq2_a(jc + 2)
                if jc == 2:
                    sq1_compute()

                m2r_cur = m2r_tiles.pop(jc)
                for mt in range(MT):
                    if len(pending_red) >= 2:
                        pending_red.pop(0)()
                    pdt = psD.tile([P, CHUNK], F32, tag="psD")
                    nc.tensor.matmul(
                        pdt[:],
                        laug[:, mt * P : (mt + 1) * P],
                        raug[:, jc * CHUNK : (jc + 1) * CHUNK],
                        start=True,
                        stop=False,
                    )
                    for ks in range(KS):
                        nc.tensor.matmul(
                            pdt[:],
                            m1r[ks][:, mt],
                            m2r_cur[ks][:],
                            start=False,
                            stop=(ks == KS - 1),
                            perf_mode=DRM,
                        )
                    pending_red.append(
                        lambda mt_=mt, pdt_=pdt, jc_=jc: reduce(mt_, pdt_, jc_)
                    )

                # next chunk's sq2 rows enter raug after this chunk's matmuls
                # so the PE queue never stalls on them mid-chunk
                if jc + 1 < NJC:
                    sq2_b(jc + 1, hls_q.pop(jc + 1))

            for r in pending_red:
                r()

            # ---- finale: per-row ap/an, loss, precision; column sums ----
            pmin = statp.tile([P, MT], F32, tag="fin")
            pmax = statp.tile([P, MT], F32, tag="fin")
            for mt in range(MT):
                nc.vector.tensor_reduce(
                    pmin[:, mt : mt + 1], minb[mt][:], AX, OP.min
                )
                nc.vector.tensor_reduce(
                    pmax[:, mt : mt + 1], maxb[mt][:], AX, OP.max
                )
            # ap_sq = max(-2*pmin - BIG + sq1, EPS); an_sq = max(-2*pmax + sq1, EPS)
            apq = statp.tile([P, MT], F32, tag="fin")
            nc.vector.tensor_scalar(apq[:], pmin[:], -2.0, BIG, OP.mult, OP.subtract)
            nc.vector.tensor_tensor(apq[:], apq[:], sq1c[:], OP.add)
            apq2 = statp.tile([P, MT], F32, tag="fin")
            nc.vector.tensor_scalar(apq2[:], apq[:], EPS, None, OP.max)
            anq = statp.tile([P, MT], F32, tag="fin")
            nc.vector.tensor_scalar(anq[:], pmax[:], -2.0, None, OP.mult)
            nc.vector.tensor_tensor(anq[:], anq[:], sq1c[:], OP.add)
            nc.vector.tensor_scalar(anq[:], anq[:], EPS, None, OP.max)

            prec = statp.tile([P, MT], F32, tag="fin")
            nc.vector.tensor_tensor(prec[:], anq[:], apq2[:], OP.is_gt)

            ap = statp.tile([P, MT], F32, tag="fin")
            nc.scalar.activation(ap[:], apq2[:], AF.Sqrt)
            an = statp.tile([P, MT], F32, tag="fin")
            nc.scalar.activation(an[:], anq[:], AF.Sqrt)

            lp = statp.tile([P, 2 * MT], F32, tag="fin2")
            nc.vector.tensor_sub(lp[:, 0:MT], ap[:], an[:])
            nc.vector.tensor_scalar(
                lp[:, 0:MT], lp[:, 0:MT], margin, 0.0, OP.add, OP.max
            )
            nc.vector.tensor_copy(lp[:, MT : 2 * MT], prec[:])

            pf = psS.tile([2 * MT, 1], F32, tag="psS")
            nc.tensor.matmul(pf[:], lp[:], ones_col[:])
            osb = statp.tile([2 * MT, 1], F32, tag="fin")
            nc.vector.tensor_copy(osb[:], pf[:])
            nc.sync.dma_start(out_d[:, :], osb[:])

    nc.finalize()
    return nc


@functools.lru_cache(maxsize=4)
def _get_program(margin: float) -> bass.Bass:
    return _build(margin)


def _make_in_maps(m1q, m2q, tgt_f32):
    iden = np.eye(P, dtype=np.float32)
    iota = np.arange(NIDS, dtype=np.float32).reshape(NIDS, 1)
    # m2 DR layout: [ks*128+p, jc*1024 + i*512 + jj] = m2q[jc*512+jj, ks*256+i*128+p]
    m2r = np.ascontiguousarray(
        m2q.T.reshape(KS, 2, P, NJC, CHUNK)
        .transpose(0, 2, 3, 1, 4)
        .reshape(KS * P, NJC * 1024)
    )
    maps = []
    for c in range(NCORES):
        r = m1q[c * SH : (c + 1) * SH]
        # m1 DR layout: [ks*128+p, mt*256 + i*128 + m] = r[mt*128+m, ks*256+i*128+p]
        m1r = np.ascontiguousarray(
            r.T.reshape(KS, 2, P, MT, P)
            .transpose(0, 2, 3, 1, 4)
            .reshape(KS * P, MT * 256)
        )
        maps.append(
            {
                "m1r": m1r,
                "m2r": m2r,
                "m1n": r,
                "m2n": m2q,
                "tgt": tgt_f32,
                "tgts": np.ascontiguousarray(tgt_f32[:, c * SH : (c + 1) * SH]),
                "iden": iden,
                "iota": iota,
            }
        )
    return maps


def run(modal1_inputs, modal2_inputs, targets, margin, trace=False):
    m1q = np.asarray(modal1_inputs, dtype=np.float32).astype(E4)
    m2q = np.asarray(modal2_inputs, dtype=np.float32).astype(E4)
    tgt_f32 = np.asarray(targets).astype(np.float32).astype(BF).reshape(1, N)
    nc = _get_program(float(margin))
    res = run_bass_kernel_spmd(
        nc, _make_in_maps(m1q, m2q, tgt_f32), list(range(NCORES)), trace=trace
    )
    loss_sum = 0.0
    prec_sum = 0.0
    for r in res.results:
        o = r["out"].reshape(-1)
        loss_sum += float(o[:MT].sum())
        prec_sum += float(o[MT:].sum())
    loss = np.float32(loss_sum / N)
    prec = np.float32(prec_sum / N)
    return (loss, prec), res


def kernel(modal1_inputs, modal2_inputs, targets, margin):
    (loss, prec), _ = run(modal1_inputs, modal2_inputs, targets, margin)
    return loss, prec


# revision 16
# speedup vs baseline: 1.4260x; 1.0696x over previous
"""Cross-modal triplet loss (margin ranking on hardest pos/neg pairs) on 8 trn2 NeuronCores.

Strategy: shard rows of modal1 across the 8 cores (512 rows each); replicate
modal2 and targets. Inputs are quantized to fp8 e4m3 on the host and shipped in
two layouts: K-major DoubleRow layout (two 128-row K-subtiles side by side) so
the PE runs fp8 DoubleRow matmuls at 2x bf16 rate with no on-chip transposes,
and natural-layout bf16 (exact widening of the same fp8 values) for row-norm
computation via scalar-engine Square+accumulate.

Per (m-tile, chunk) PSUM group:
    psum[m, j] = dot(m1q[m], m2q[j]) - sq2[j]/2 - (BIG/2) * mask[m, j]
computed as one bf16 "aug" matmul (66 contraction rows: same-identity one-hot
mask over 64 ids, and the hi/lo bf16 split of -sq2/2) followed by 8 fp8
DoubleRow matmuls (K=2048). The aug matmul leads the group so the group close
never waits on sq2 production, which is software-pipelined one chunk ahead.
h = -2*psum = sq2 - 2g + BIG*mask, so the row-wise psum max/min give
hardest-negative / (BIG + hardest-positive) squared distances up to the row
constant sq1[m], added after the reductions in fp32. sqrt only on the final
per-row values. Per-core loss/precision partials are combined on the host.

Distances are exact metrics on the fp8-quantized vectors (norms computed from
the same quantized values the matmul sees): loss error ~7e-4 relative vs the
2e-2 gate; precision stays exactly 0 (min row gap an-ap ~ 4.2 >> fp8 noise).
"""

import functools

import ml_dtypes
import numpy as np

import concourse.bass as bass
import concourse.mybir as mybir
import concourse.tile as tile
from concourse import bacc
from concourse.bass_utils import run_bass_kernel_spmd

F32 = mybir.dt.float32
BF16 = mybir.dt.bfloat16
FP8 = mybir.dt.float8e4
E4 = ml_dtypes.float8_e4m3
BF = ml_dtypes.bfloat16
OP = mybir.AluOpType
AF = mybir.ActivationFunctionType
AX = mybir.AxisListType.X
DRM = mybir.MatmulPerfMode.DoubleRow

N, D, NIDS, P = 4096, 2048, 64, 128
NCORES = 8
SH = N // NCORES      # 512 rows of modal1 per core
MT = SH // P          # 4 m-tiles per core
KS = D // 256         # 8 DoubleRow K-supertiles (256 contraction rows each)
CHUNK = 512           # modal2 columns per chunk (one PSUM bank of fp32)
NJC = N // CHUNK      # 8 chunks
JTC = CHUNK // P      # 4 j-tiles per chunk
KAUG = 66             # one-hot mask (0:64), sq2 hi/lo (64:66)
BIG = 16384.0         # > max dist_sq (~6500); exact in bf16
EPS = 1e-12


def _build(margin: float) -> bass.Bass:
    nc = bacc.Bacc(num_swdge_queues=4)
    m1r_d = nc.dram_tensor("m1r", [KS * P, MT * 256], FP8, kind="ExternalInput")
    m2r_d = nc.dram_tensor("m2r", [KS * P, NJC * 1024], FP8, kind="ExternalInput")
    m1n_d = nc.dram_tensor("m1n", [SH, D], FP8, kind="ExternalInput")
    m2n_d = nc.dram_tensor("m2n", [N, D], FP8, kind="ExternalInput")
    tgt_d = nc.dram_tensor("tgt", [1, N], BF16, kind="ExternalInput")
    tgts_d = nc.dram_tensor("tgts", [1, SH], BF16, kind="ExternalInput")
    iden_d = nc.dram_tensor("iden", [P, P], F32, kind="ExternalInput")
    iota_d = nc.dram_tensor("iota", [NIDS, 1], F32, kind="ExternalInput")
    out_d = nc.dram_tensor("out", [2 * MT, 1], F32, kind="ExternalOutput")

    with tile.TileContext(nc) as tc:
        with (
            tc.tile_pool(name="const", bufs=1) as const,
            tc.tile_pool(name="m1rp", bufs=KS) as m1rp,
            tc.tile_pool(name="m2rp", bufs=2 * KS) as m2rp,
            tc.tile_pool(name="natp", bufs=12) as natp,
            tc.tile_pool(name="scr", bufs=4) as scrp,
            tc.tile_pool(name="small", bufs=12) as smallp,
            tc.tile_pool(name="stat", bufs=2 * MT + 10) as statp,
            tc.tile_pool(name="psD", bufs=7, space=bass.MemorySpace.PSUM) as psD,
            tc.tile_pool(name="psS", bufs=1, space=bass.MemorySpace.PSUM) as psS,
        ):
            # ---- constants ----
            iden = const.tile([P, P], F32)
            nc.sync.dma_start(iden[:], iden_d[:, :])
            iota_f = const.tile([NIDS, 1], F32)
            nc.sync.dma_start(iota_f[:], iota_d[:, :])
            ones_col = const.tile([P, 1], F32)
            nc.vector.memset(ones_col[:], 1.0)

            # ---- aug lhsT [66, SH]: -BIG/2 * onehot(own ids); rows 64:66 = 1 ----
            laug = const.tile([KAUG, SH], BF16)
            bc1 = const.tile([NIDS, SH], BF16)
            nc.sync.dma_start(bc1[:], tgts_d[0:1, :].broadcast_to((NIDS, SH)))
            nc.vector.tensor_scalar(
                laug[0:NIDS, :], bc1[:], iota_f[:], -BIG / 2.0, OP.is_equal, OP.mult
            )
            nc.gpsimd.memset(laug[NIDS:KAUG, :], 1.0)

            # ---- aug rhs [66, N]: onehot(all ids); rows 64:66 filled per chunk ----
            raug = const.tile([KAUG, N], BF16)
            bc2 = const.tile([NIDS, N], BF16)
            nc.gpsimd.dma_start(bc2[:], tgt_d[0:1, :].broadcast_to((NIDS, N)))
            nc.vector.tensor_scalar(
                raug[0:NIDS, :], bc2[:], iota_f[:], None, OP.is_equal
            )

            # ---- m1 DR-layout supertiles (gpsimd queues, after chunk-0 rhs) ----
            m1r = []
            for ks in range(KS):
                t = m1rp.tile([P, MT, 2, P], FP8, tag="m1r", name=f"m1r{ks}")
                nc.gpsimd.dma_start(t[:], m1r_d[ks * P : (ks + 1) * P, :])
                m1r.append(t)

            sq1c = const.tile([P, MT], F32)

            def sq1_compute():
                # issued mid-loop: only needed by the finale
                for mt in range(MT):
                    t = natp.tile([P, D], FP8, tag="m1nat")
                    nc.sync.dma_start(t[:], m1n_d[mt * P : (mt + 1) * P, :])
                    scr = scrp.tile([P, D], BF16, tag="scr")
                    nc.scalar.activation(
                        scr[:], t[:], AF.Square, accum_out=sq1c[:, mt : mt + 1]
                    )

            # ---- running per-row min/max of psum over chunks ----
            minb = [statp.tile([P, NJC], F32, tag="stat", name=f"minb{i}") for i in range(MT)]
            maxb = [statp.tile([P, NJC], F32, tag="stat", name=f"maxb{i}") for i in range(MT)]

            # ---- software-pipelined sq2 production (one chunk ahead) ----
            # phase A: DMA natural bf16 j-tiles, Square+accum, hi/lo split
            # phase B (after current chunk's matmuls in the PE queue):
            #          PE-transpose hi/lo into raug rows 64:66
            def sq2_a(jc):
                sqcols = smallp.tile([P, JTC], F32, tag="sqc")
                for jt in range(JTC):
                    j0 = jc * JTC + jt
                    t = natp.tile([P, D], FP8, tag="m2nat")
                    nc.sync.dma_start(t[:], m2n_d[j0 * P : (j0 + 1) * P, :])
                    scr = scrp.tile([P, D], BF16, tag="scr")
                    on_vec = jt == 3 or (jc < 2 and jt >= 2)
                    if on_vec:
                        nc.vector.tensor_tensor(scr[:], t[:], t[:], OP.mult)
                        nc.vector.tensor_reduce(
                            sqcols[:, jt : jt + 1], scr[:], AX, OP.add
                        )
                    else:
                        nc.scalar.activation(
                            scr[:], t[:], AF.Square,
                            accum_out=sqcols[:, jt : jt + 1],
                        )
                v = smallp.tile([P, JTC], F32, tag="sqv")
                nc.vector.tensor_scalar(v[:], sqcols[:], -0.5, None, OP.mult)
                hb = smallp.tile([P, JTC], BF16, tag="hb")
                nc.vector.tensor_copy(hb[:], v[:])
                hl = smallp.tile([P, JTC, 2], F32, tag="hl")
                nc.vector.tensor_copy(hl[:, :, 0:1], hb[:])
                nc.vector.tensor_sub(hl[:, :, 1:2], v[:], hl[:, :, 0:1])
                return hl

            def sq2_b(jc, hl):
                pS = psS.tile([2, JTC * P], F32, tag="psS")
                for jt in range(JTC):
                    nc.tensor.transpose(
                        pS[:, jt * P : (jt + 1) * P], hl[:, jt], iden[:]
                    )
                nc.vector.tensor_copy(
                    raug[NIDS : NIDS + 2, jc * CHUNK : (jc + 1) * CHUNK], pS[:]
                )

            def m2r_load(jc):
                tiles = []
                for ks in range(KS):
                    t = m2rp.tile([P, 2, CHUNK], FP8, tag="m2r")
                    eng = nc.sync if ks >= 6 else nc.gpsimd
                    eng.dma_start(
                        t[:],
                        m2r_d[ks * P : (ks + 1) * P, jc * 1024 : (jc + 1) * 1024],
                    )
                    tiles.append(t)
                return tiles

            def reduce(mt, pdt, jc_):
                nc.vector.tensor_reduce(
                    minb[mt][:, jc_ : jc_ + 1], pdt[:], AX, OP.min
                )
                nc.vector.tensor_reduce(
                    maxb[mt][:, jc_ : jc_ + 1], pdt[:], AX, OP.max
                )

            # preamble: sq2 production runs two chunks ahead of consumption;
            # raug transposes land one chunk ahead (between matmul blocks)
            m2r_tiles = {0: m2r_load(0)}
            hls_q = {0: sq2_a(0), 1: sq2_a(1)}
            sq2_b(0, hls_q.pop(0))

            pending_red = []
            for jc in range(NJC):
                if jc + 1 < NJC:
                    m2r_tiles[jc + 1] = m2r_load(jc + 1)
                    sq2_b(jc + 1, hls_q.pop(jc + 1))
                if jc + 2 < NJC:
                    hls_q[jc + 2] = sq2_a(jc + 2)
                if jc == 2:
                    sq1_compute()

                m2r_cur = m2r_tiles.pop(jc)
                for mt in range(MT):
                    if len(pending_red) >= 2:
                        pending_red.pop(0)()
                    pdt = psD.tile([P, CHUNK], F32, tag="psD")
                    nc.tensor.matmul(
                        pdt[:],
                        laug[:, mt * P : (mt + 1) * P],
                        raug[:, jc * CHUNK : (jc + 1) * CHUNK],
                        start=True,
                        stop=False,
                    )
                    for ks in range(KS):
                        nc.tensor.matmul(
                            pdt[:],
                            m1r[ks][:, mt],
                            m2r_cur[ks][:],
                            start=False,
                            stop=(ks == KS - 1),
                            perf_mode=DRM,
                        )
                    pending_red.append(
                        lambda mt_=mt, pdt_=pdt, jc_=jc: reduce(mt_, pdt_, jc_)
                    )



            for r in pending_red:
                r()

            # ---- finale: per-row ap/an, loss, precision; column sums ----
            pmin = statp.tile([P, MT], F32, tag="fin")
            pmax = statp.tile([P, MT], F32, tag="fin")
            for mt in range(MT):
                nc.vector.tensor_reduce(
                    pmin[:, mt : mt + 1], minb[mt][:], AX, OP.min
                )
                nc.vector.tensor_reduce(
                    pmax[:, mt : mt + 1], maxb[mt][:], AX, OP.max
                )
            # ap_sq = max(-2*pmin - BIG + sq1, EPS); an_sq = max(-2*pmax + sq1, EPS)
            apq = statp.tile([P, MT], F32, tag="fin")
            nc.vector.tensor_scalar(apq[:], pmin[:], -2.0, BIG, OP.mult, OP.subtract)
            nc.vector.tensor_tensor(apq[:], apq[:], sq1c[:], OP.add)
            apq2 = statp.tile([P, MT], F32, tag="fin")
            nc.vector.tensor_scalar(apq2[:], apq[:], EPS, None, OP.max)
            anq = statp.tile([P, MT], F32, tag="fin")
            nc.vector.tensor_scalar(anq[:], pmax[:], -2.0, None, OP.mult)
            nc.vector.tensor_tensor(anq[:], anq[:], sq1c[:], OP.add)
            nc.vector.tensor_scalar(anq[:], anq[:], EPS, None, OP.max)

            prec = statp.tile([P, MT], F32, tag="fin")
            nc.vector.tensor_tensor(prec[:], anq[:], apq2[:], OP.is_gt)

            ap = statp.tile([P, MT], F32, tag="fin")
            nc.scalar.activation(ap[:], apq2[:], AF.Sqrt)
            an = statp.tile([P, MT], F32, tag="fin")
            nc.scalar.activation(an[:], anq[:], AF.Sqrt)

            lp = statp.tile([P, 2 * MT], F32, tag="fin2")
            nc.vector.tensor_sub(lp[:, 0:MT], ap[:], an[:])
            nc.vector.tensor_scalar(
                lp[:, 0:MT], lp[:, 0:MT], margin, 0.0, OP.add, OP.max
            )
            nc.vector.tensor_copy(lp[:, MT : 2 * MT], prec[:])

            pf = psS.tile([2 * MT, 1], F32, tag="psS")
            nc.tensor.matmul(pf[:], lp[:], ones_col[:])
            osb = statp.tile([2 * MT, 1], F32, tag="fin")
            nc.vector.tensor_copy(osb[:], pf[:])
            nc.sync.dma_start(out_d[:, :], osb[:])

    nc.finalize()
    return nc


@functools.lru_cache(maxsize=4)
def _get_program(margin: float) -> bass.Bass:
    return _build(margin)


def _make_in_maps(m1q, m2q, tgt_f32):
    iden = np.eye(P, dtype=np.float32)
    iota = np.arange(NIDS, dtype=np.float32).reshape(NIDS, 1)
    # m2 DR layout: [ks*128+p, jc*1024 + i*512 + jj] = m2q[jc*512+jj, ks*256+i*128+p]
    m2r = np.ascontiguousarray(
        m2q.T.reshape(KS, 2, P, NJC, CHUNK)
        .transpose(0, 2, 3, 1, 4)
        .reshape(KS * P, NJC * 1024)
    )
    maps = []
    for c in range(NCORES):
        r = m1q[c * SH : (c + 1) * SH]
        # m1 DR layout: [ks*128+p, mt*256 + i*128 + m] = r[mt*128+m, ks*256+i*128+p]
        m1r = np.ascontiguousarray(
            r.T.reshape(KS, 2, P, MT, P)
            .transpose(0, 2, 3, 1, 4)
            .reshape(KS * P, MT * 256)
        )
        maps.append(
            {
                "m1r": m1r,
                "m2r": m2r,
                "m1n": r,
                "m2n": m2q,
                "tgt": tgt_f32,
                "tgts": np.ascontiguousarray(tgt_f32[:, c * SH : (c + 1) * SH]),
                "iden": iden,
                "iota": iota,
            }
        )
    return maps


def run(modal1_inputs, modal2_inputs, targets, margin, trace=False):
    m1q = np.asarray(modal1_inputs, dtype=np.float32).astype(E4)
    m2q = np.asarray(modal2_inputs, dtype=np.float32).astype(E4)
    tgt_f32 = np.asarray(targets).astype(np.float32).astype(BF).reshape(1, N)
    nc = _get_program(float(margin))
    res = run_bass_kernel_spmd(
        nc, _make_in_maps(m1q, m2q, tgt_f32), list(range(NCORES)), trace=trace
    )
    loss_sum = 0.0
    prec_sum = 0.0
    for r in res.results:
        o = r["out"].reshape(-1)
        loss_sum += float(o[:MT].sum())
        prec_sum += float(o[MT:].sum())
    loss = np.float32(loss_sum / N)
    prec = np.float32(prec_sum / N)
    return (loss, prec), res


def kernel(modal1_inputs, modal2_inputs, targets, margin):
    (loss, prec), _ = run(modal1_inputs, modal2_inputs, targets, margin)
    return loss, prec


# revision 17
# speedup vs baseline: 1.5114x; 1.0599x over previous
"""Cross-modal triplet loss (margin ranking on hardest pos/neg pairs) on 8 trn2 NeuronCores.

Strategy: shard rows of modal1 across the 8 cores (512 rows each); replicate
modal2 and targets. Inputs are quantized to fp8 e4m3 on the host and shipped in
two layouts: K-major DoubleRow layout (two 128-row K-subtiles side by side) so
the PE runs fp8 DoubleRow matmuls at 2x bf16 rate with no on-chip transposes,
and natural-layout bf16 (exact widening of the same fp8 values) for row-norm
computation via scalar-engine Square+accumulate.

Per (m-tile, chunk) PSUM group:
    psum[m, j] = dot(m1q[m], m2q[j]) - sq2[j]/2 - (BIG/2) * mask[m, j]
computed as one bf16 "aug" matmul (66 contraction rows: same-identity one-hot
mask over 64 ids, and the hi/lo bf16 split of -sq2/2) followed by 8 fp8
DoubleRow matmuls (K=2048). The aug matmul leads the group so the group close
never waits on sq2 production, which is software-pipelined one chunk ahead.
h = -2*psum = sq2 - 2g + BIG*mask, so the row-wise psum max/min give
hardest-negative / (BIG + hardest-positive) squared distances up to the row
constant sq1[m], added after the reductions in fp32. sqrt only on the final
per-row values. Per-core loss/precision partials are combined on the host.

Distances are exact metrics on the fp8-quantized vectors (norms computed from
the same quantized values the matmul sees): loss error ~7e-4 relative vs the
2e-2 gate; precision stays exactly 0 (min row gap an-ap ~ 4.2 >> fp8 noise).
"""

import functools

import ml_dtypes
import numpy as np

import concourse.bass as bass
import concourse.mybir as mybir
import concourse.tile as tile
from concourse import bacc
from concourse.bass_utils import run_bass_kernel_spmd

F32 = mybir.dt.float32
BF16 = mybir.dt.bfloat16
FP8 = mybir.dt.float8e4
E4 = ml_dtypes.float8_e4m3
BF = ml_dtypes.bfloat16
OP = mybir.AluOpType
AF = mybir.ActivationFunctionType
AX = mybir.AxisListType.X
DRM = mybir.MatmulPerfMode.DoubleRow

N, D, NIDS, P = 4096, 2048, 64, 128
NCORES = 8
SH = N // NCORES      # 512 rows of modal1 per core
MT = SH // P          # 4 m-tiles per core
KS = D // 256         # 8 DoubleRow K-supertiles (256 contraction rows each)
CHUNK = 512           # modal2 columns per chunk (one PSUM bank of fp32)
NJC = N // CHUNK      # 8 chunks
JTC = CHUNK // P      # 4 j-tiles per chunk
KAUG = 66             # one-hot mask (0:64), sq2 hi/lo (64:66)
BIG = 16384.0         # > max dist_sq (~6500); exact in bf16
EPS = 1e-12


def _build(margin: float) -> bass.Bass:
    nc = bacc.Bacc(num_swdge_queues=4)
    m1r_d = nc.dram_tensor("m1r", [KS * P, MT * 256], FP8, kind="ExternalInput")
    m2r_d = nc.dram_tensor("m2r", [KS * P, NJC * 1024], FP8, kind="ExternalInput")
    m1n_d = nc.dram_tensor("m1n", [SH, D], FP8, kind="ExternalInput")
    m2n_d = nc.dram_tensor("m2n", [N, D], FP8, kind="ExternalInput")
    m2nb_d = nc.dram_tensor("m2nb", [NJC * P, D], BF16, kind="ExternalInput")
    tgt_d = nc.dram_tensor("tgt", [1, N], BF16, kind="ExternalInput")
    tgts_d = nc.dram_tensor("tgts", [1, SH], BF16, kind="ExternalInput")
    iden_d = nc.dram_tensor("iden", [P, P], F32, kind="ExternalInput")
    iota_d = nc.dram_tensor("iota", [NIDS, 1], F32, kind="ExternalInput")
    out_d = nc.dram_tensor("out", [2 * MT, 1], F32, kind="ExternalOutput")

    with tile.TileContext(nc) as tc:
        with (
            tc.tile_pool(name="const", bufs=1) as const,
            tc.tile_pool(name="m1rp", bufs=KS) as m1rp,
            tc.tile_pool(name="m2rp", bufs=2 * KS) as m2rp,
            tc.tile_pool(name="natp", bufs=12) as natp,
            tc.tile_pool(name="scr", bufs=4) as scrp,
            tc.tile_pool(name="small", bufs=12) as smallp,
            tc.tile_pool(name="stat", bufs=2 * MT + 10) as statp,
            tc.tile_pool(name="psD", bufs=7, space=bass.MemorySpace.PSUM) as psD,
            tc.tile_pool(name="psS", bufs=1, space=bass.MemorySpace.PSUM) as psS,
        ):
            # ---- constants ----
            iden = const.tile([P, P], F32)
            nc.sync.dma_start(iden[:], iden_d[:, :])
            iota_f = const.tile([NIDS, 1], F32)
            nc.sync.dma_start(iota_f[:], iota_d[:, :])
            ones_col = const.tile([P, 1], F32)
            nc.vector.memset(ones_col[:], 1.0)

            # ---- aug lhsT [66, SH]: -BIG/2 * onehot(own ids); rows 64:66 = 1 ----
            laug = const.tile([KAUG, SH], BF16)
            bc1 = const.tile([NIDS, SH], BF16)
            nc.sync.dma_start(bc1[:], tgts_d[0:1, :].broadcast_to((NIDS, SH)))
            nc.vector.tensor_scalar(
                laug[0:NIDS, :], bc1[:], iota_f[:], -BIG / 2.0, OP.is_equal, OP.mult
            )
            nc.gpsimd.memset(laug[NIDS:KAUG, :], 1.0)

            # ---- aug rhs [66, N]: onehot(all ids); rows 64:66 filled per chunk ----
            raug = const.tile([KAUG, N], BF16)
            bc2 = const.tile([NIDS, N], BF16)
            nc.gpsimd.dma_start(bc2[:], tgt_d[0:1, :].broadcast_to((NIDS, N)))
            nc.vector.tensor_scalar(
                raug[0:NIDS, :], bc2[:], iota_f[:], None, OP.is_equal
            )

            # ---- m1 DR-layout supertiles (gpsimd queues, after chunk-0 rhs) ----
            m1r = []
            for ks in range(KS):
                t = m1rp.tile([P, MT, 2, P], FP8, tag="m1r", name=f"m1r{ks}")
                nc.gpsimd.dma_start(t[:], m1r_d[ks * P : (ks + 1) * P, :])
                m1r.append(t)

            sq1c = const.tile([P, MT], F32)

            def sq1_compute():
                # issued mid-loop: only needed by the finale
                for mt in range(MT):
                    t = natp.tile([P, D], FP8, tag="m1nat")
                    nc.sync.dma_start(t[:], m1n_d[mt * P : (mt + 1) * P, :])
                    scr = scrp.tile([P, D], BF16, tag="scr")
                    nc.scalar.activation(
                        scr[:], t[:], AF.Square, accum_out=sq1c[:, mt : mt + 1]
                    )

            # ---- running per-row min/max of psum over chunks ----
            minb = [statp.tile([P, NJC], F32, tag="stat", name=f"minb{i}") for i in range(MT)]
            maxb = [statp.tile([P, NJC], F32, tag="stat", name=f"maxb{i}") for i in range(MT)]

            # ---- software-pipelined sq2 production (one chunk ahead) ----
            # phase A: DMA natural bf16 j-tiles, Square+accum, hi/lo split
            # phase B (after current chunk's matmuls in the PE queue):
            #          PE-transpose hi/lo into raug rows 64:66
            def sq2_a(jc):
                sqcols = smallp.tile([P, JTC], F32, tag="sqc")
                for jt in range(JTC):
                    j0 = jc * JTC + jt
                    scr = scrp.tile([P, D], BF16, tag="scr")
                    if jt == 3:
                        # bf16 copy of this block: DVE runs 16-bit at 2x
                        tb = natp.tile([P, D], BF16, tag="m2natb")
                        nc.sync.dma_start(
                            tb[:], m2nb_d[jc * P : (jc + 1) * P, :]
                        )
                        nc.vector.tensor_tensor(scr[:], tb[:], tb[:], OP.mult)
                        nc.vector.tensor_reduce(
                            sqcols[:, jt : jt + 1], scr[:], AX, OP.add
                        )
                    else:
                        t = natp.tile([P, D], FP8, tag="m2nat")
                        nc.sync.dma_start(t[:], m2n_d[j0 * P : (j0 + 1) * P, :])
                        nc.scalar.activation(
                            scr[:], t[:], AF.Square,
                            accum_out=sqcols[:, jt : jt + 1],
                        )
                v = smallp.tile([P, JTC], F32, tag="sqv")
                nc.vector.tensor_scalar(v[:], sqcols[:], -0.5, None, OP.mult)
                hb = smallp.tile([P, JTC], BF16, tag="hb")
                nc.vector.tensor_copy(hb[:], v[:])
                hl = smallp.tile([P, JTC, 2], F32, tag="hl")
                nc.vector.tensor_copy(hl[:, :, 0:1], hb[:])
                nc.vector.tensor_sub(hl[:, :, 1:2], v[:], hl[:, :, 0:1])
                return hl

            def sq2_b(jc, hl):
                pS = psS.tile([2, JTC * P], F32, tag="psS")
                for jt in range(JTC):
                    nc.tensor.transpose(
                        pS[:, jt * P : (jt + 1) * P], hl[:, jt], iden[:]
                    )
                nc.vector.tensor_copy(
                    raug[NIDS : NIDS + 2, jc * CHUNK : (jc + 1) * CHUNK], pS[:]
                )

            def m2r_load(jc):
                tiles = []
                for ks in range(KS):
                    t = m2rp.tile([P, 2, CHUNK], FP8, tag="m2r")
                    eng = nc.sync if ks >= 6 else nc.gpsimd
                    eng.dma_start(
                        t[:],
                        m2r_d[ks * P : (ks + 1) * P, jc * 1024 : (jc + 1) * 1024],
                    )
                    tiles.append(t)
                return tiles

            def reduce(mt, pdt, jc_):
                nc.vector.tensor_reduce(
                    minb[mt][:, jc_ : jc_ + 1], pdt[:], AX, OP.min
                )
                nc.vector.tensor_reduce(
                    maxb[mt][:, jc_ : jc_ + 1], pdt[:], AX, OP.max
                )

            # preamble: sq2 production runs two chunks ahead of consumption;
            # raug transposes land one chunk ahead (between matmul blocks)
            m2r_tiles = {0: m2r_load(0)}
            hls_q = {0: sq2_a(0), 1: sq2_a(1)}
            sq2_b(0, hls_q.pop(0))

            pending_red = []
            for jc in range(NJC):
                if jc + 1 < NJC:
                    m2r_tiles[jc + 1] = m2r_load(jc + 1)
                    sq2_b(jc + 1, hls_q.pop(jc + 1))
                if jc + 2 < NJC:
                    hls_q[jc + 2] = sq2_a(jc + 2)
                if jc == 2:
                    sq1_compute()

                m2r_cur = m2r_tiles.pop(jc)
                for mt in range(MT):
                    if len(pending_red) >= 2:
                        pending_red.pop(0)()
                    pdt = psD.tile([P, CHUNK], F32, tag="psD")
                    nc.tensor.matmul(
                        pdt[:],
                        laug[:, mt * P : (mt + 1) * P],
                        raug[:, jc * CHUNK : (jc + 1) * CHUNK],
                        start=True,
                        stop=False,
                    )
                    for ks in range(KS):
                        nc.tensor.matmul(
                            pdt[:],
                            m1r[ks][:, mt],
                            m2r_cur[ks][:],
                            start=False,
                            stop=(ks == KS - 1),
                            perf_mode=DRM,
                        )
                    pending_red.append(
                        lambda mt_=mt, pdt_=pdt, jc_=jc: reduce(mt_, pdt_, jc_)
                    )



            for r in pending_red:
                r()

            # ---- finale: per-row ap/an, loss, precision; column sums ----
            pmin = statp.tile([P, MT], F32, tag="fin")
            pmax = statp.tile([P, MT], F32, tag="fin")
            for mt in range(MT):
                nc.vector.tensor_reduce(
                    pmin[:, mt : mt + 1], minb[mt][:], AX, OP.min
                )
                nc.vector.tensor_reduce(
                    pmax[:, mt : mt + 1], maxb[mt][:], AX, OP.max
                )
            # ap_sq = max(-2*pmin - BIG + sq1, EPS); an_sq = max(-2*pmax + sq1, EPS)
            apq = statp.tile([P, MT], F32, tag="fin")
            nc.vector.tensor_scalar(apq[:], pmin[:], -2.0, BIG, OP.mult, OP.subtract)
            nc.vector.tensor_tensor(apq[:], apq[:], sq1c[:], OP.add)
            apq2 = statp.tile([P, MT], F32, tag="fin")
            nc.vector.tensor_scalar(apq2[:], apq[:], EPS, None, OP.max)
            anq = statp.tile([P, MT], F32, tag="fin")
            nc.vector.tensor_scalar(anq[:], pmax[:], -2.0, None, OP.mult)
            nc.vector.tensor_tensor(anq[:], anq[:], sq1c[:], OP.add)
            nc.vector.tensor_scalar(anq[:], anq[:], EPS, None, OP.max)

            prec = statp.tile([P, MT], F32, tag="fin")
            nc.vector.tensor_tensor(prec[:], anq[:], apq2[:], OP.is_gt)

            ap = statp.tile([P, MT], F32, tag="fin")
            nc.scalar.activation(ap[:], apq2[:], AF.Sqrt)
            an = statp.tile([P, MT], F32, tag="fin")
            nc.scalar.activation(an[:], anq[:], AF.Sqrt)

            lp = statp.tile([P, 2 * MT], F32, tag="fin2")
            nc.vector.tensor_sub(lp[:, 0:MT], ap[:], an[:])
            nc.vector.tensor_scalar(
                lp[:, 0:MT], lp[:, 0:MT], margin, 0.0, OP.add, OP.max
            )
            nc.vector.tensor_copy(lp[:, MT : 2 * MT], prec[:])

            pf = psS.tile([2 * MT, 1], F32, tag="psS")
            nc.tensor.matmul(pf[:], lp[:], ones_col[:])
            osb = statp.tile([2 * MT, 1], F32, tag="fin")
            nc.vector.tensor_copy(osb[:], pf[:])
            nc.sync.dma_start(out_d[:, :], osb[:])

    nc.finalize()
    return nc


@functools.lru_cache(maxsize=4)
def _get_program(margin: float) -> bass.Bass:
    return _build(margin)


def _make_in_maps(m1q, m2q, tgt_f32):
    iden = np.eye(P, dtype=np.float32)
    iota = np.arange(NIDS, dtype=np.float32).reshape(NIDS, 1)
    # m2 DR layout: [ks*128+p, jc*1024 + i*512 + jj] = m2q[jc*512+jj, ks*256+i*128+p]
    m2r = np.ascontiguousarray(
        m2q.T.reshape(KS, 2, P, NJC, CHUNK)
        .transpose(0, 2, 3, 1, 4)
        .reshape(KS * P, NJC * 1024)
    )
    m2nb = np.ascontiguousarray(
        m2q.reshape(NJC, JTC, P, D)[:, 3].reshape(NJC * P, D).astype(BF)
    )
    maps = []
    for c in range(NCORES):
        r = m1q[c * SH : (c + 1) * SH]
        # m1 DR layout: [ks*128+p, mt*256 + i*128 + m] = r[mt*128+m, ks*256+i*128+p]
        m1r = np.ascontiguousarray(
            r.T.reshape(KS, 2, P, MT, P)
            .transpose(0, 2, 3, 1, 4)
            .reshape(KS * P, MT * 256)
        )
        maps.append(
            {
                "m1r": m1r,
                "m2r": m2r,
                "m1n": r,
                "m2n": m2q,
                "m2nb": m2nb,
                "tgt": tgt_f32,
                "tgts": np.ascontiguousarray(tgt_f32[:, c * SH : (c + 1) * SH]),
                "iden": iden,
                "iota": iota,
            }
        )
    return maps


def run(modal1_inputs, modal2_inputs, targets, margin, trace=False):
    m1q = np.asarray(modal1_inputs, dtype=np.float32).astype(E4)
    m2q = np.asarray(modal2_inputs, dtype=np.float32).astype(E4)
    tgt_f32 = np.asarray(targets).astype(np.float32).astype(BF).reshape(1, N)
    nc = _get_program(float(margin))
    res = run_bass_kernel_spmd(
        nc, _make_in_maps(m1q, m2q, tgt_f32), list(range(NCORES)), trace=trace
    )
    loss_sum = 0.0
    prec_sum = 0.0
    for r in res.results:
        o = r["out"].reshape(-1)
        loss_sum += float(o[:MT].sum())
        prec_sum += float(o[MT:].sum())
    loss = np.float32(loss_sum / N)
    prec = np.float32(prec_sum / N)
    return (loss, prec), res


def kernel(modal1_inputs, modal2_inputs, targets, margin):
    (loss, prec), _ = run(modal1_inputs, modal2_inputs, targets, margin)
    return loss, prec
